# revision 11
# baseline (speedup 1.0000x reference)
"""Trainium2 Bass kernel for nn_EquiformerV2Conv (gnn_message_passing).

Math: per-edge rotations R cancel against R^T around the per-l channel mix,
so the network reduces to
    G   = segment_sum(x[src], dst)
    ew  = mean_e sigmoid(h(d_e) + pb),  h(d) = cut(d)*sum_k pw_k gauss_k(d)
    out = [ silu(layernorm(G0 @ W0)) | (ew/sqrt(32)) * per-xyz G1 @ W1 ]
(LayerNorm is scale-invariant, so ew and the 1/sqrt(64)/E factors drop from
the l=0 path.)

Device dataflow per core (8-way edge partition, x replicated):
  - HYBRID gather: edge-slot blocks are typed E/O (dma_gather of 1024B
    pair-rows, idx=src>>1, lo/hi operand slice by src parity) or I
    (indirect qPoolDynamic DMA of 320B x rows, int32 idx, IBK*128 rows per
    call).  dma_gather desc-gen (~7.4ns/idx) runs on the Pool Q7 while the
    indirect ring drains concurrently (~1.1us per call), so the two gather
    channels overlap.
  - scatter-add = S^T @ X matmuls into per-window PSUM; transpose to
    channel-major; W-mix FUSED into the transpose-back (rhs=W0 / ew*W1
    instead of the identity); LayerNorm (Rsqrt) + SiLU on the l=0 slice.
  - ew: fp32 Clenshaw of a host-fitted deg-11 Chebyshev of h(d) on
    w=0.4d-1, Sigmoid+accumulate on Scalar, cross-partition sum via
    ones-matmul, 8-core AllReduce; consumed at kc=3 (chunks 0-2 deferred),
    folded into the l=1 mix weights.
"""
import os
import numpy as np
import ml_dtypes

bf16 = ml_dtypes.bfloat16
f32 = np.float32

# problem constants
N = 50000
E = 400000
SC, VC, DIM, NB = 64, 32, 160, 64
CUTOFF, EPS = 5.0, 1e-5

# distribution constants
P = 128            # slots per block
W = 51             # windows per core
WSLOTS = 1024      # slots per window
BLKW = WSLOTS // P          # 8 blocks per window
NBLK = W * BLKW             # 408
SLOTS = NBLK * P            # 52224
GW = 3                      # windows per pipeline chunk
NKC = W // GW               # 17
ELEM2 = 512                 # bf16 elements per pair-row (1024 B, 2 nodes)
H2 = ELEM2 // 2
OUTROWS = W * P             # 6528
NCOEF = 12                  # Chebyshev coefficients (deg 11)
NCORES = 8

# ---- gather-channel split knobs ------------------------------------------
# per 3-window chunk: block types for each window (8 blocks each).
# 'E': dma_gather, even-src edges (lo slice); 'O': odd-src (hi slice);
# 'I': indirect DMA (any parity).
IBK = 1  # indirect rows per call / 128

_WIN_PATTERNS = [
    list("EEEOOIII"),   # w%3==0: 3E 2O 3I
    list("EEOOOIII"),   # w%3==1: 2E 3O 3I
    list("EEOOIIII"),   # w%3==2: 2E 2O 4I
]


def _chunk_layout():
    """Per-chunk block layout: list over 3 windows of type strings, plus
    flattened lists of gather blocks and indirect blocks (wi, bpos)."""
    wins = [_WIN_PATTERNS[w % 3] for w in range(3)]
    gblocks = [(wi, b) for wi in range(3) for b in range(BLKW)
               if wins[wi][b] in "EO"]
    iblocks = [(wi, b) for wi in range(3) for b in range(BLKW)
               if wins[wi][b] == "I"]
    return wins, gblocks, iblocks


_WINS3, _GBLOCKS, _IBLOCKS = _chunk_layout()
NGB = len(_GBLOCKS)            # gather blocks per chunk
NIB = len(_IBLOCKS)            # indirect blocks per chunk
assert NIB % IBK == 0
NIC = NIB // IBK               # indirect calls per chunk
GIDX_N = NGB * P               # gather idx per chunk
GIDX_C = GIDX_N // 16          # gidx columns per chunk


def _win_caps(w):
    pat = _WIN_PATTERNS[w % 3]
    return (pat.count("E") * P, pat.count("O") * P, pat.count("I") * P)


# ---------------------------------------------------------------- host side

def _pack_nodes(degE, degO):
    """Assign nodes to (window, slot) respecting per-window class caps.
    degE/degO: per dst node count of even-src / odd-src in-edges."""
    order = np.argsort(-(degE + degO), kind="stable")
    nbins = NCORES * W
    caps = [_win_caps(k // NCORES) for k in range(nbins)]
    loadE = np.zeros(nbins, np.int64)
    loadO = np.zeros(nbins, np.int64)
    cnt = np.zeros(nbins, np.int64)
    node2win = np.full(N, -1, np.int64)
    node2slot = np.full(N, -1, np.int64)
    start = 0
    for n in order:
        a, b = degE[n], degO[n]
        for k in range(nbins):
            wbin = (start + k) % nbins
            cE, cO, cI = caps[wbin]
            E2 = loadE[wbin] + a
            O2 = loadO[wbin] + b
            if (E2 <= cE + cI and O2 <= cO + cI
                    and E2 + O2 <= cE + cO + cI and cnt[wbin] < P):
                node2win[n] = wbin
                node2slot[n] = cnt[wbin]
                loadE[wbin] = E2
                loadO[wbin] = O2
                cnt[wbin] += 1
                start = (wbin + 1) % nbins
                break
        else:
            raise RuntimeError(f"window packing failed at node {n}")
    return node2win, node2slot


def _fit_poly(cent, wid, pw):
    """Chebyshev fit of h(d) = cut(d)*sum_k pw_k gauss_k(d) on w = 0.4d-1."""
    dg = np.linspace(0.0, CUTOFF, 6000)
    g = np.exp(-0.5 * ((dg[:, None] - cent) / wid) ** 2)
    cut = 0.5 * (np.cos(np.pi * dg / CUTOFF) + 1.0)
    h = (g @ pw) * cut
    ser = np.polynomial.chebyshev.Chebyshev.fit(0.4 * dg - 1.0, h, NCOEF - 1,
                                                domain=[-1, 1])
    return ser.coef.astype(f32)


def _stage(x, pos, src, dst):
    """Build all per-core device input arrays."""
    isE = (src & 1) == 0
    degE = np.bincount(dst[isE], minlength=N)
    degO = np.bincount(dst[~isE], minlength=N)
    node2win, node2slot = _pack_nodes(degE, degO)
    win_core = node2win % NCORES
    win_local = node2win // NCORES

    # pair-rows: [x[2k] 160 | pad | x[2k+1] 160 | pad], l1 cols j-major
    perm = np.arange(DIM)
    l1 = np.arange(SC, DIM)
    cc = (l1 - SC) // 3
    jj = (l1 - SC) % 3
    perm[SC + 32 * jj + cc] = l1
    xr = x[:, perm].astype(bf16)
    xp2 = np.zeros((N // 2, ELEM2), bf16)
    xp2[:, 0:DIM] = xr[0::2]
    xp2[:, H2:H2 + DIM] = xr[1::2]

    e_core = win_core[dst]
    e_wl = win_local[dst]
    e_slot = node2slot[dst]
    e_isO = (src & 1).astype(np.int64)

    dvec = (pos[src] - pos[dst]).astype(np.float64)
    de = np.sqrt((dvec ** 2).sum(1))
    we = (np.minimum(0.4 * de, 2.0) - 1.0).astype(f32)

    # per-window block maps (local window index -> block lists)
    blists = {}
    for wl in range(W):
        pat = _WIN_PATTERNS[wl % 3]
        blists[wl] = dict(
            E=[b for b in range(BLKW) if pat[b] == "E"],
            O=[b for b in range(BLKW) if pat[b] == "O"],
            I=[b for b in range(BLKW) if pat[b] == "I"],
        )

    # group edges by (core, local window, parity); within-group rank
    key = (e_core * W + e_wl) * 2 + e_isO
    order = np.argsort(key, kind="stable")
    ks = key[order]
    grp_start = np.searchsorted(ks, np.arange(2 * NCORES * W + 1))
    within = np.arange(E) - grp_start[ks]

    ins = []
    meta_edges = []
    for r in range(NCORES):
        # slot arrays for this core
        slot_src = np.zeros(SLOTS, np.int64)     # src node (for gidx/i32)
        slot_dst = np.full(SLOTS, -1, np.int64)  # dst slot within window
        slot_w = np.ones(SLOTS, f32)
        n_edges = 0
        for wl in range(W):
            bl = blists[wl]
            capE, capO = len(bl["E"]) * P, len(bl["O"]) * P
            gE = 2 * (r * W + wl)
            eidx_E = order[grp_start[gE]:grp_start[gE + 1]]
            eidx_O = order[grp_start[gE + 1]:grp_start[gE + 2]]
            n_edges += len(eidx_E) + len(eidx_O)
            # fill E blocks then overflow to I; same for O
            iPool = []
            for eidx, blocks, cap in ((eidx_E, bl["E"], capE),
                                      (eidx_O, bl["O"], capO)):
                take = min(len(eidx), cap)
                for i in range(take):
                    b = blocks[i // P]
                    s = wl * WSLOTS + b * P + i % P
                    e = eidx[i]
                    slot_src[s] = src[e]
                    slot_dst[s] = e_slot[e]
                    slot_w[s] = we[e]
                iPool.extend(eidx[take:])
            assert len(iPool) <= len(bl["I"]) * P, (r, wl, len(iPool))
            for i, e in enumerate(iPool):
                b = bl["I"][i // P]
                s = wl * WSLOTS + b * P + i % P
                slot_src[s] = src[e]
                slot_dst[s] = e_slot[e]
                slot_w[s] = we[e]
        meta_edges.append(n_edges)

        # build gidx (dma_gather idx, pair indices, wrapped 16) per chunk
        gidx = np.zeros((128, NKC * GIDX_C), np.int16)
        i32 = np.zeros((128, NKC * NIC * IBK), np.int32)
        for kc in range(NKC):
            glist = []
            for (wi, b) in _GBLOCKS:
                wl = kc * GW + wi
                s0 = wl * WSLOTS + b * P
                glist.append(slot_src[s0:s0 + P] >> 1)
            gi = np.concatenate(glist).astype(np.int16)       # [NGB*P]
            blk = gi.reshape(GIDX_N // 16, 16).T              # [16, cols]
            gidx[:, kc * GIDX_C:(kc + 1) * GIDX_C] = np.tile(blk, (8, 1))
            for ci in range(NIC):
                cols = []
                for (wi, b) in _IBLOCKS[ci * IBK:(ci + 1) * IBK]:
                    wl = kc * GW + wi
                    s0 = wl * WSLOTS + b * P
                    cols.append(slot_src[s0:s0 + P])
                c0 = (kc * NIC + ci) * IBK
                i32[:, c0:c0 + IBK] = np.stack(cols, axis=1)

        wcl = slot_w.reshape(NBLK, P).T.copy()                # [128, NBLK]

        smat = np.zeros((NBLK, P, P), bf16)                   # [blk, edge_p, slot]
        blk = np.arange(SLOTS) // P
        pp = np.arange(SLOTS) % P
        valid = slot_dst >= 0
        smat[blk[valid], pp[valid], slot_dst[valid]] = bf16(1.0)
        smat = smat.transpose(1, 0, 2).reshape(P, NBLK * P)

        ins.append(dict(gidx=gidx, gidx0=gidx[:, :GIDX_C].copy(),
                        i32=i32, wcl=wcl, smat=smat))

    meta = dict(node2win=node2win, node2slot=node2slot,
                win_core=win_core, win_local=win_local, n_edges=meta_edges)
    return xp2, xr.copy(), ins, meta


# ---------------------------------------------------------------- device side

_PROG = None


def _build_program():
    import concourse.bacc as bacc
    import concourse.tile as tile
    from concourse import mybir, library_config, bass

    dt = mybir.dt
    Alu = mybir.AluOpType
    Act = mybir.ActivationFunctionType

    nc = bacc.Bacc("TRN2", target_bir_lowering=False, debug=False,
                   num_devices=NCORES)

    xp2_d = nc.dram_tensor("xp2", [N // 2, ELEM2], dt.bfloat16, kind="ExternalInput")
    xrow_d = nc.dram_tensor("xrow", [N, DIM], dt.bfloat16, kind="ExternalInput")
    gidx0_d = nc.dram_tensor("gidx0", [P, GIDX_C], dt.int16, kind="ExternalInput")
    gidx_d = nc.dram_tensor("gidx", [P, NKC * GIDX_C], dt.int16, kind="ExternalInput")
    i32_d = nc.dram_tensor("i32", [P, NKC * NIC * IBK], dt.int32, kind="ExternalInput")
    smat_d = nc.dram_tensor("smat", [P, SLOTS], dt.bfloat16, kind="ExternalInput")
    wcl_d = nc.dram_tensor("wcl", [P, NBLK], dt.float32, kind="ExternalInput")
    coef_d = nc.dram_tensor("coef", [P, NCOEF], dt.float32, kind="ExternalInput")
    corr_d = nc.dram_tensor("corr", [1, 1], dt.float32, kind="ExternalInput")
    pbb_d = nc.dram_tensor("pbb", [P, 1], dt.float32, kind="ExternalInput")
    w0_d = nc.dram_tensor("w0", [SC, SC], dt.bfloat16, kind="ExternalInput")
    w1_d = nc.dram_tensor("w1", [VC, VC], dt.bfloat16, kind="ExternalInput")
    w1f_d = nc.dram_tensor("w1f", [VC, VC], dt.float32, kind="ExternalInput")
    identb_d = nc.dram_tensor("identb", [P, P], dt.bfloat16, kind="ExternalInput")
    gamr_d = nc.dram_tensor("gamr", [P, SC], dt.float32, kind="ExternalInput")
    betr_d = nc.dram_tensor("betr", [P, SC], dt.float32, kind="ExternalInput")
    onesc_d = nc.dram_tensor("onesc", [P, 1], dt.float32, kind="ExternalInput")
    onesr_d = nc.dram_tensor("onesr", [1, P], dt.float32, kind="ExternalInput")
    out_d = nc.dram_tensor("out", [OUTROWS, DIM], dt.float32, kind="ExternalOutput")

    s1 = float(1.0 / np.sqrt(VC) / E)

    with tile.TileContext(nc, num_cores=NCORES) as tc:
        import contextlib
        with contextlib.ExitStack() as ctx:
            consts = ctx.enter_context(tc.tile_pool(name="consts", bufs=1))
            gbuf = ctx.enter_context(tc.tile_pool(name="gbuf", bufs=1))
            gather = ctx.enter_context(tc.tile_pool(name="gather", bufs=2))
            ipool = ctx.enter_context(tc.tile_pool(name="ipool", bufs=2))
            spool = ctx.enter_context(tc.tile_pool(name="spool", bufs=2))
            obp = ctx.enter_context(tc.tile_pool(name="obp", bufs=2))
            gcp = ctx.enter_context(tc.tile_pool(name="gcp", bufs=6))
            ocp = ctx.enter_context(tc.tile_pool(name="ocp", bufs=6))
            zpool = ctx.enter_context(tc.tile_pool(name="zpool", bufs=3))
            lnp = ctx.enter_context(tc.tile_pool(name="lnp", bufs=4))
            psG = ctx.enter_context(tc.tile_pool(name="psG", bufs=2, space="PSUM"))
            psT = ctx.enter_context(tc.tile_pool(name="psT", bufs=2, space="PSUM"))
            psT2 = ctx.enter_context(tc.tile_pool(name="psT2", bufs=2, space="PSUM"))
            psE = ctx.enter_context(tc.tile_pool(name="psE", bufs=1, space="PSUM"))
            dram = ctx.enter_context(tc.tile_pool(name="dram", bufs=1, space="DRAM"))

            nc.gpsimd.load_library(library_config.mlp)

            # ---- constant loads (gidx0 first: the kc=0 gather needs only it)
            gidx0 = consts.tile([P, GIDX_C], dt.int16)
            nc.sync.dma_start(out=gidx0[:], in_=gidx0_d[:])
            i32 = consts.tile([P, NKC * NIC * IBK], dt.int32)
            nc.sync.dma_start(out=i32[:], in_=i32_d[:])
            gidx = consts.tile([P, NKC * GIDX_C], dt.int16)
            nc.sync.dma_start(out=gidx[:], in_=gidx_d[:])
            wcl = consts.tile([P, NBLK], dt.float32)
            nc.sync.dma_start(out=wcl[:], in_=wcl_d[:])
            coef = consts.tile([P, NCOEF], dt.float32)
            nc.sync.dma_start(out=coef[:], in_=coef_d[:])
            corrt = consts.tile([1, 1], dt.float32)
            nc.sync.dma_start(out=corrt[:], in_=corr_d[:])
            pbb = consts.tile([P, 1], dt.float32)
            nc.sync.dma_start(out=pbb[:], in_=pbb_d[:])
            w0sb = consts.tile([SC, SC], dt.bfloat16)
            nc.sync.dma_start(out=w0sb[:], in_=w0_d[:])
            w1sb = consts.tile([VC, VC], dt.bfloat16)
            nc.sync.dma_start(out=w1sb[:], in_=w1_d[:])
            w1f = consts.tile([VC, VC], dt.float32)
            nc.sync.dma_start(out=w1f[:], in_=w1f_d[:])
            identb = consts.tile([P, P], dt.bfloat16)
            nc.sync.dma_start(out=identb[:], in_=identb_d[:])
            gamr = consts.tile([P, SC], dt.float32)
            nc.sync.dma_start(out=gamr[:], in_=gamr_d[:])
            betr = consts.tile([P, SC], dt.float32)
            nc.sync.dma_start(out=betr[:], in_=betr_d[:])
            onesc = consts.tile([P, 1], dt.float32)
            nc.sync.dma_start(out=onesc[:], in_=onesc_d[:])
            onesr = consts.tile([1, P], dt.float32)
            nc.sync.dma_start(out=onesr[:], in_=onesr_d[:])

            # ---- z-phase: Clenshaw of deg-11 Chebyshev (emitted up front;
            # runs on Vector while the first gathers proceed)
            w2c = gbuf.tile([P, NBLK], dt.float32)
            nc.vector.tensor_scalar(out=w2c[:], in0=wcl[:], scalar1=2.0,
                                    scalar2=None, op0=Alu.mult)
            b1 = zpool.tile([P, NBLK], dt.float32, tag="zb")
            nc.vector.tensor_scalar(out=b1[:], in0=wcl[:], scalar1=0.0,
                                    scalar2=coef[:, NCOEF - 1:NCOEF],
                                    op0=Alu.mult, op1=Alu.add)
            b2 = zpool.tile([P, NBLK], dt.float32, tag="zb")
            nc.vector.memset(b2[:], 0.0)
            for k in range(NCOEF - 2, 0, -1):
                t = zpool.tile([P, NBLK], dt.float32, tag="zt")
                nc.vector.tensor_tensor(out=t[:], in0=w2c[:], in1=b1[:],
                                        op=Alu.mult)
                bn = zpool.tile([P, NBLK], dt.float32, tag="zb")
                nc.vector.scalar_tensor_tensor(
                    out=bn[:], in0=t[:], scalar=coef[:, k:k + 1], in1=b2[:],
                    op0=Alu.add, op1=Alu.subtract)
                b2, b1 = b1, bn
            tf = zpool.tile([P, NBLK], dt.float32, tag="zt")
            nc.vector.tensor_tensor(out=tf[:], in0=wcl[:], in1=b1[:], op=Alu.mult)
            uz = gbuf.tile([P, NBLK], dt.float32)
            nc.vector.scalar_tensor_tensor(
                out=uz[:], in0=tf[:], scalar=coef[:, 0:1], in1=b2[:],
                op0=Alu.add, op1=Alu.subtract)
            zscr = gbuf.tile([P, NBLK], dt.float32)
            zsum = gbuf.tile([P, 1], dt.float32)

            ewc1 = gbuf.tile([P, 1], dt.float32)
            w1ew = gbuf.tile([VC, VC], dt.bfloat16)
            outv = out_d[:].rearrange("(w p) d -> p w d", p=P)

            def process_chunk(pkc, gpk):
                """Fused mix(+transpose-back) + LN + SiLU + store for chunk
                pkc.  Requires w1ew (ew-folded W1) to be emitted already."""
                och = ocp.tile([P, GW, DIM], dt.float32, tag="och", name="och")
                for wi in range(GW):
                    lo = wi * P
                    t2pk = psT2.tile([P, DIM], dt.float32, tag="t2pk", name="t2pk")
                    nc.tensor.matmul(t2pk[:, 0:SC], gpk[:, 0, lo:lo + P],
                                     w0sb[:], start=True, stop=True)
                    for j in range(3):
                        nc.tensor.matmul(t2pk[:, SC + VC * j:SC + VC * (j + 1)],
                                         gpk[0:VC, 1 + j, lo:lo + P],
                                         w1ew[:], start=True, stop=True)
                    nc.scalar.copy(out=och[:, wi, :], in_=t2pk[:])

                ob0 = och[:, :, 0:SC]
                mu = lnp.tile([P, GW], dt.float32, tag="mu", name="mu")
                nc.vector.tensor_reduce(out=mu[:], in_=ob0,
                                        axis=mybir.AxisListType.X, op=Alu.add)
                mu2 = lnp.tile([P, GW], dt.float32, tag="mu2", name="mu2")
                nc.vector.tensor_scalar(out=mu2[:], in0=mu[:],
                                        scalar1=float(1.0 / SC), scalar2=None,
                                        op0=Alu.mult)
                cen = lnp.tile([P, GW, SC], dt.float32, tag="cen", name="cen")
                nc.vector.tensor_tensor(
                    out=cen[:], in0=ob0,
                    in1=mu2[:].unsqueeze(2).to_broadcast([P, GW, SC]),
                    op=Alu.subtract)
                sq = lnp.tile([P, GW, SC], dt.float32, tag="lnt", name="sq")
                nc.vector.tensor_tensor(out=sq[:], in0=cen[:], in1=cen[:],
                                        op=Alu.mult)
                varb = lnp.tile([P, GW], dt.float32, tag="mu", name="varb")
                nc.vector.tensor_reduce(out=varb[:], in_=sq[:],
                                        axis=mybir.AxisListType.X, op=Alu.add)
                vb2 = lnp.tile([P, GW], dt.float32, tag="mu2", name="vb2")
                nc.vector.tensor_scalar(out=vb2[:], in0=varb[:],
                                        scalar1=float(1.0 / SC), scalar2=float(EPS),
                                        op0=Alu.mult, op1=Alu.add)
                sdb = lnp.tile([P, GW], dt.float32, tag="mu", name="sdb")
                nc.scalar.activation(out=sdb[:], in_=vb2[:], func=Act.Sqrt)
                rsb = lnp.tile([P, GW], dt.float32, tag="mu3", name="rsb")
                nc.vector.reciprocal(out=rsb[:], in_=sdb[:])
                t1b_ = lnp.tile([P, GW, SC], dt.float32, tag="lnt", name="t1b_")
                nc.vector.tensor_tensor(
                    out=t1b_[:], in0=cen[:],
                    in1=rsb[:].unsqueeze(2).to_broadcast([P, GW, SC]),
                    op=Alu.mult)
                t2b_ = lnp.tile([P, GW, SC], dt.float32, tag="lnt", name="t2b_")
                nc.vector.tensor_tensor(
                    out=t2b_[:], in0=t1b_[:],
                    in1=gamr[:].unsqueeze(1).to_broadcast([P, GW, SC]),
                    op=Alu.mult)
                t3b_ = lnp.tile([P, GW, SC], dt.float32, tag="lnt", name="t3b_")
                nc.vector.tensor_tensor(
                    out=t3b_[:], in0=t2b_[:],
                    in1=betr[:].unsqueeze(1).to_broadcast([P, GW, SC]),
                    op=Alu.add)
                nc.scalar.activation(out=ob0, in_=t3b_[:], func=Act.Silu)
                nc.scalar.dma_start(out=outv[:, pkc * GW:(pkc + 1) * GW, :],
                                    in_=och[:])

            pending = []
            prev = None
            for kc in range(NKC):
                sload = spool.tile([P, GW * BLKW, P], dt.bfloat16, tag="sload")
                nc.sync.dma_start(
                    out=sload[:],
                    in_=smat_d[:, kc * GW * BLKW * P:(kc + 1) * GW * BLKW * P])
                # indirect calls first: the dynamic ring drains while the
                # dma_gather desc-gen runs on the Pool Q7
                xi = []
                for ci in range(NIC):
                    xt = ipool.tile([P, IBK * DIM], dt.bfloat16, tag=f"xi{ci}")
                    c0 = (kc * NIC + ci) * IBK
                    nc.gpsimd.indirect_dma_start(
                        out=xt[:], out_offset=None, in_=xrow_d[:, :],
                        in_offset=bass.IndirectOffsetOnAxis(
                            ap=i32[:, c0:c0 + IBK], axis=0))
                    xi.append(xt)
                xg = gather.tile([P, NGB, ELEM2], dt.bfloat16, tag="xg")
                idxs = gidx0[:] if kc == 0 else \
                    gidx[:, kc * GIDX_C:(kc + 1) * GIDX_C]
                nc.gpsimd.dma_gather(
                    xg[:], xp2_d[:, :], idxs, GIDX_N, GIDX_N,
                    ELEM2, single_packet=False)

                obw = obp.tile([P, GW, DIM], dt.bfloat16, tag="obw")
                gpk = gcp.tile([SC, 1 + 3, GW * P], dt.bfloat16, tag="gpk")
                gpos = {gb: j for j, gb in enumerate(_GBLOCKS)}
                ipos = {ib: j for j, ib in enumerate(_IBLOCKS)}
                for wi in range(GW):
                    pat = _WIN_PATTERNS[(kc * GW + wi) % 3]
                    gps = psG.tile([P, DIM], dt.float32, tag="gps")
                    for b in range(BLKW):
                        ty = pat[b]
                        if ty == "I":
                            j = ipos[(wi, b)]
                            ci, sub = j // IBK, j % IBK
                            operand = xi[ci][:, sub * DIM:(sub + 1) * DIM]
                        else:
                            j = gpos[(wi, b)]
                            sl = slice(0, DIM) if ty == "E" else \
                                slice(H2, H2 + DIM)
                            operand = xg[:, j, sl]
                        nc.tensor.matmul(
                            gps[:], sload[:, wi * BLKW + b, :], operand,
                            start=(b == 0), stop=(b == BLKW - 1))
                    nc.scalar.copy(out=obw[:, wi, :], in_=gps[:])
                    # transpose to channel-major
                    tpk = psT.tile([SC, 4, P], dt.bfloat16, tag="tpk")
                    nc.tensor.transpose(out=tpk[:, 0, :], in_=obw[:, wi, 0:SC],
                                        identity=identb[:])
                    for j in range(3):
                        nc.tensor.transpose(
                            out=tpk[0:VC, 1 + j, :],
                            in_=obw[:, wi, SC + VC * j:SC + VC * (j + 1)],
                            identity=identb[:])
                    nc.scalar.copy(out=gpk[:, :, wi * P:(wi + 1) * P], in_=tpk[:])

                if kc == 1:
                    # z-phase tail after kc0/1 copies flowed through Scalar
                    nc.scalar.activation(out=zscr[:], in_=uz[:], func=Act.Sigmoid,
                                         bias=pbb[:, 0:1], accum_out=zsum[:, 0:1])
                    ewt = psE.tile([P, 2], dt.float32, tag="ewt", name="ewt")
                    nc.tensor.matmul(ewt[0:1, 0:1], onesc[:], zsum[:],
                                     start=True, stop=True)
                    zc8 = gbuf.tile([1, 8], dt.float32)
                    nc.vector.tensor_scalar(
                        out=zc8[:], in0=ewt[0:1, 0:1].to_broadcast([1, 8]),
                        scalar1=corrt[0:1, 0:1], scalar2=None, op0=Alu.subtract)
                    arin = dram.tile([1, 8], dt.float32)
                    arout = dram.tile([1, 8], dt.float32)
                    nc.sync.dma_start(out=arin[:], in_=zc8[:])
                    nc.gpsimd.collective_compute(
                        "AllReduce", Alu.add, replica_groups=[list(range(NCORES))],
                        ins=[arin.opt()], outs=[arout.opt()])

                if kc == 3:
                    ewsb = gbuf.tile([1, 8], dt.float32)
                    nc.sync.dma_start(out=ewsb[:], in_=arout[:])
                    ewg = gbuf.tile([1, 1], dt.float32)
                    nc.vector.tensor_scalar(out=ewg[:], in0=ewsb[0:1, 0:1],
                                            scalar1=s1, scalar2=None, op0=Alu.mult)
                    nc.tensor.matmul(ewt[:, 1:2], onesr[:], ewg[:],
                                     start=True, stop=True)
                    nc.scalar.copy(out=ewc1[:], in_=ewt[:, 1:2])
                    w1s = gbuf.tile([VC, VC], dt.float32)
                    nc.vector.tensor_scalar(out=w1s[:], in0=w1f[:],
                                            scalar1=ewc1[0:VC, 0:1],
                                            scalar2=None, op0=Alu.mult)
                    nc.vector.tensor_copy(out=w1ew[:], in_=w1s[:])
                    for pkc, pgpk in pending:
                        process_chunk(pkc, pgpk)
                    pending.clear()

                if kc < 3:
                    pending.append((kc, gpk))
                else:
                    if prev is not None:
                        process_chunk(*prev)
                    prev = (kc, gpk)

            process_chunk(*prev)

    nc.compile()
    return nc


def _get_program():
    global _PROG
    if _PROG is None:
        _PROG = _build_program()
    return _PROG


# ---------------------------------------------------------------- entry point

def kernel(**inputs):
    from concourse.bass_utils import run_bass_kernel_spmd

    x = np.asarray(inputs["x"], f32)
    pos = np.asarray(inputs["pos"], f32)
    ei = np.asarray(inputs["edge_index"])
    src = ei[0].astype(np.int64)
    dst = ei[1].astype(np.int64)

    xp2, xrow, cores, meta = _stage(x, pos, src, dst)

    cent = np.asarray(inputs["rbf_centers"], np.float64).reshape(-1)
    wid = np.asarray(inputs["rbf_widths"], np.float64).reshape(-1)
    pw = np.asarray(inputs["edge_proj_w"], np.float64).reshape(-1)
    pb = float(np.asarray(inputs["edge_proj_b"]).reshape(-1)[0])
    coefs = _fit_poly(cent, wid, pw)
    sig_pb = 1.0 / (1.0 + np.exp(-pb))

    common = dict(
        xp2=xp2,
        xrow=xrow,
        coef=np.tile(coefs[None, :], (P, 1)).astype(f32),
        pbb=np.full((P, 1), pb, f32),
        w0=np.asarray(inputs["W0"], f32).astype(bf16),
        w1=np.asarray(inputs["W1"], f32).astype(bf16),
        w1f=np.asarray(inputs["W1"], f32),
        identb=np.eye(P, dtype=bf16),
        gamr=np.tile(np.asarray(inputs["ln_gamma"], f32).reshape(1, SC), (P, 1)),
        betr=np.tile(np.asarray(inputs["ln_beta"], f32).reshape(1, SC), (P, 1)),
        onesc=np.ones((P, 1), f32),
        onesr=np.ones((1, P), f32),
    )
    in_maps = []
    for r in range(NCORES):
        cd = cores[r]
        n_empty = SLOTS - meta["n_edges"][r]
        in_maps.append(dict(
            common, gidx=cd["gidx"], gidx0=cd["gidx0"], i32=cd["i32"],
            wcl=cd["wcl"], smat=cd["smat"],
            corr=np.array([[n_empty * sig_pb]], f32)))

    nc = _get_program()
    trace = bool(int(os.environ.get("KERNEL_TRACE", "0")))
    res = run_bass_kernel_spmd(nc, in_maps, core_ids=list(range(NCORES)),
                               trace=trace)
    kernel.last_results = res

    # assemble full output
    out_full = np.zeros((N, DIM), f32)
    col_map = np.arange(DIM)
    for jj in range(3):
        for cc in range(VC):
            col_map[SC + 3 * cc + jj] = SC + VC * jj + cc
    n2w, n2s = meta["node2win"], meta["node2slot"]
    wc, wl = meta["win_core"], meta["win_local"]
    for r in range(NCORES):
        o = res.results[r]["out"]                      # [W*P, DIM]
        nodes = np.nonzero(wc == r)[0]
        rows = wl[nodes] * P + n2s[nodes]
        out_full[nodes] = o[rows][:, col_map]
    return out_full


# revision 19
# speedup vs baseline: 1.6439x; 1.6439x over previous
"""Trainium2 Bass kernel for nn_EquiformerV2Conv (gnn_message_passing).

Math: per-edge rotations R cancel against R^T around the per-l channel mix,
so the network reduces to
    G   = segment_sum(x[src], dst)
    ew  = mean_e sigmoid(h(d_e) + pb),  h(d) = cut(d)*sum_k pw_k gauss_k(d)
    out = [ silu(layernorm(G0 @ W0)) | (ew/sqrt(32)) * per-xyz G1 @ W1 ]
(LayerNorm is scale-invariant, so ew and the 1/sqrt(64)/E factors drop from
the l=0 path.)

Device dataflow per core (8-way edge partition, x replicated):
  - HYBRID gather: edge-slot blocks are typed E/O (dma_gather of 1024B
    pair-rows, idx=src>>1, lo/hi operand slice by src parity) or I
    (indirect qPoolDynamic DMA of 320B x rows, int32 idx, IBK*128 rows per
    call).  dma_gather desc-gen (~7.4ns/idx) runs on the Pool Q7 while the
    indirect ring drains concurrently (~1.1us per call), so the two gather
    channels overlap.
  - scatter-add = S^T @ X matmuls into per-window PSUM; transpose to
    channel-major; W-mix FUSED into the transpose-back (rhs=W0 / ew*W1
    instead of the identity); LayerNorm (Rsqrt) + SiLU on the l=0 slice.
  - ew: fp32 Clenshaw of a host-fitted deg-11 Chebyshev of h(d) on
    w=0.4d-1, Sigmoid+accumulate on Scalar, cross-partition sum via
    ones-matmul, 8-core AllReduce; consumed at kc=3 (chunks 0-2 deferred),
    folded into the l=1 mix weights.
"""
import os
import numpy as np
import ml_dtypes

bf16 = ml_dtypes.bfloat16
f32 = np.float32

# problem constants
N = 50000
E = 400000
SC, VC, DIM, NB = 64, 32, 160, 64
CUTOFF, EPS = 5.0, 1e-5

# distribution constants
P = 128            # slots per block
W = 51             # windows per core
WSLOTS = 1024      # slots per window
BLKW = WSLOTS // P          # 8 blocks per window
NBLK = W * BLKW             # 408
SLOTS = NBLK * P            # 52224
GW = 3                      # windows per pipeline chunk
NKC = W // GW               # 17
ELEM2 = 512                 # bf16 elements per pair-row (1024 B, 2 nodes)
H2 = ELEM2 // 2
OUTROWS = W * P             # 6528
NCOEF = 12                  # Chebyshev coefficients (deg 11)
NCORES = 8

# ---- gather-channel split knobs ------------------------------------------
# per 3-window chunk: block types for each window (8 blocks each).
# 'E': dma_gather, even-src edges (lo slice); 'O': odd-src (hi slice);
# 'I': indirect DMA (any parity).
IBK = 1  # indirect rows per call / 128
NQ = 4   # SWDGE queues; per-chunk dma_gather split into NQ parallel calls

_WIN_PATTERNS = [
    list("EEEEOOOO"),
    list("EEEEOOOO"),
    list("EEEEOOOO"),
]


def _chunk_layout():
    """Per-chunk block layout: list over 3 windows of type strings, plus
    flattened lists of gather blocks and indirect blocks (wi, bpos)."""
    wins = [_WIN_PATTERNS[w % 3] for w in range(3)]
    gblocks = [(wi, b) for wi in range(3) for b in range(BLKW)
               if wins[wi][b] in "EO"]
    iblocks = [(wi, b) for wi in range(3) for b in range(BLKW)
               if wins[wi][b] == "I"]
    return wins, gblocks, iblocks


_WINS3, _GBLOCKS, _IBLOCKS = _chunk_layout()
NGB = len(_GBLOCKS)            # gather blocks per chunk
NIB = len(_IBLOCKS)            # indirect blocks per chunk
assert NIB % IBK == 0
NIC = NIB // IBK               # indirect calls per chunk
assert NGB % NQ == 0
GBQ = NGB // NQ                # gather blocks per queue-call
GIDX_N = NGB * P               # gather idx per chunk
GIDX_NQ = GBQ * P              # gather idx per queue-call
GIDX_CQ = GIDX_NQ // 16        # gidx columns per queue-call
GIDX_C = GIDX_N // 16          # gidx columns per chunk


def _win_caps(w):
    pat = _WIN_PATTERNS[w % 3]
    return (pat.count("E") * P, pat.count("O") * P, pat.count("I") * P)


# ---------------------------------------------------------------- host side

def _pack_nodes(degE, degO):
    """Assign nodes to (window, slot) respecting per-window class caps.
    degE/degO: per dst node count of even-src / odd-src in-edges."""
    order = np.argsort(-(degE + degO), kind="stable")
    nbins = NCORES * W
    caps = [_win_caps(k // NCORES) for k in range(nbins)]
    loadE = np.zeros(nbins, np.int64)
    loadO = np.zeros(nbins, np.int64)
    cnt = np.zeros(nbins, np.int64)
    node2win = np.full(N, -1, np.int64)
    node2slot = np.full(N, -1, np.int64)
    start = 0
    for n in order:
        a, b = degE[n], degO[n]
        for k in range(nbins):
            wbin = (start + k) % nbins
            cE, cO, cI = caps[wbin]
            E2 = loadE[wbin] + a
            O2 = loadO[wbin] + b
            if (E2 <= cE + cI and O2 <= cO + cI
                    and E2 + O2 <= cE + cO + cI and cnt[wbin] < P):
                node2win[n] = wbin
                node2slot[n] = cnt[wbin]
                loadE[wbin] = E2
                loadO[wbin] = O2
                cnt[wbin] += 1
                start = (wbin + 1) % nbins
                break
        else:
            raise RuntimeError(f"window packing failed at node {n}")
    return node2win, node2slot


def _fit_poly(cent, wid, pw):
    """Chebyshev fit of h(d) = cut(d)*sum_k pw_k gauss_k(d) on w = 0.4d-1."""
    dg = np.linspace(0.0, CUTOFF, 6000)
    g = np.exp(-0.5 * ((dg[:, None] - cent) / wid) ** 2)
    cut = 0.5 * (np.cos(np.pi * dg / CUTOFF) + 1.0)
    h = (g @ pw) * cut
    ser = np.polynomial.chebyshev.Chebyshev.fit(0.4 * dg - 1.0, h, NCOEF - 1,
                                                domain=[-1, 1])
    return ser.coef.astype(f32)


def _stage(x, pos, src, dst):
    """Build all per-core device input arrays."""
    isE = (src & 1) == 0
    degE = np.bincount(dst[isE], minlength=N)
    degO = np.bincount(dst[~isE], minlength=N)
    node2win, node2slot = _pack_nodes(degE, degO)
    win_core = node2win % NCORES
    win_local = node2win // NCORES

    # pair-rows: [x[2k] 160 | pad | x[2k+1] 160 | pad], l1 cols j-major
    perm = np.arange(DIM)
    l1 = np.arange(SC, DIM)
    cc = (l1 - SC) // 3
    jj = (l1 - SC) % 3
    perm[SC + 32 * jj + cc] = l1
    xr = x[:, perm].astype(bf16)
    xp2 = np.zeros((N // 2, ELEM2), bf16)
    xp2[:, 0:DIM] = xr[0::2]
    xp2[:, H2:H2 + DIM] = xr[1::2]

    e_core = win_core[dst]
    e_wl = win_local[dst]
    e_slot = node2slot[dst]
    e_isO = (src & 1).astype(np.int64)

    dvec = (pos[src] - pos[dst]).astype(np.float64)
    de = np.sqrt((dvec ** 2).sum(1))
    we = (np.minimum(0.4 * de, 2.0) - 1.0).astype(f32)

    # per-window block maps (local window index -> block lists)
    blists = {}
    for wl in range(W):
        pat = _WIN_PATTERNS[wl % 3]
        blists[wl] = dict(
            E=[b for b in range(BLKW) if pat[b] == "E"],
            O=[b for b in range(BLKW) if pat[b] == "O"],
            I=[b for b in range(BLKW) if pat[b] == "I"],
        )

    # group edges by (core, local window, parity); within-group rank
    key = (e_core * W + e_wl) * 2 + e_isO
    order = np.argsort(key, kind="stable")
    ks = key[order]
    grp_start = np.searchsorted(ks, np.arange(2 * NCORES * W + 1))
    within = np.arange(E) - grp_start[ks]

    ins = []
    meta_edges = []
    for r in range(NCORES):
        # slot arrays for this core
        slot_src = np.zeros(SLOTS, np.int64)     # src node (for gidx/i32)
        slot_dst = np.full(SLOTS, -1, np.int64)  # dst slot within window
        slot_w = np.ones(SLOTS, f32)
        n_edges = 0
        for wl in range(W):
            bl = blists[wl]
            capE, capO = len(bl["E"]) * P, len(bl["O"]) * P
            gE = 2 * (r * W + wl)
            eidx_E = order[grp_start[gE]:grp_start[gE + 1]]
            eidx_O = order[grp_start[gE + 1]:grp_start[gE + 2]]
            n_edges += len(eidx_E) + len(eidx_O)
            # fill E blocks then overflow to I; same for O
            iPool = []
            for eidx, blocks, cap in ((eidx_E, bl["E"], capE),
                                      (eidx_O, bl["O"], capO)):
                take = min(len(eidx), cap)
                for i in range(take):
                    b = blocks[i // P]
                    s = wl * WSLOTS + b * P + i % P
                    e = eidx[i]
                    slot_src[s] = src[e]
                    slot_dst[s] = e_slot[e]
                    slot_w[s] = we[e]
                iPool.extend(eidx[take:])
            assert len(iPool) <= len(bl["I"]) * P, (r, wl, len(iPool))
            for i, e in enumerate(iPool):
                b = bl["I"][i // P]
                s = wl * WSLOTS + b * P + i % P
                slot_src[s] = src[e]
                slot_dst[s] = e_slot[e]
                slot_w[s] = we[e]
        meta_edges.append(n_edges)

        # build gidx (dma_gather idx, pair indices, wrapped 16 per queue-call)
        gidx = np.zeros((128, NKC * GIDX_C), np.int16)
        i32 = np.zeros((128, max(1, NKC * NIC * IBK)), np.int32)
        for kc in range(NKC):
            for q in range(NQ):
                glist = []
                for (wi, b) in _GBLOCKS[q * GBQ:(q + 1) * GBQ]:
                    wl = kc * GW + wi
                    s0 = wl * WSLOTS + b * P
                    glist.append(slot_src[s0:s0 + P] >> 1)
                gi = np.concatenate(glist).astype(np.int16)   # [GBQ*P]
                blk = gi.reshape(GIDX_NQ // 16, 16).T         # [16, cols]
                c0 = kc * GIDX_C + q * GIDX_CQ
                gidx[:, c0:c0 + GIDX_CQ] = np.tile(blk, (8, 1))
            for ci in range(NIC):
                cols = []
                for (wi, b) in _IBLOCKS[ci * IBK:(ci + 1) * IBK]:
                    wl = kc * GW + wi
                    s0 = wl * WSLOTS + b * P
                    cols.append(slot_src[s0:s0 + P])
                c0 = (kc * NIC + ci) * IBK
                i32[:, c0:c0 + IBK] = np.stack(cols, axis=1)

        wcl = slot_w.reshape(NBLK, P).T.copy()                # [128, NBLK]

        smat = np.zeros((NBLK, P, P), bf16)                   # [blk, edge_p, slot]
        blk = np.arange(SLOTS) // P
        pp = np.arange(SLOTS) % P
        valid = slot_dst >= 0
        smat[blk[valid], pp[valid], slot_dst[valid]] = bf16(1.0)
        smat = smat.transpose(1, 0, 2).reshape(P, NBLK * P)

        ins.append(dict(gidx=gidx, gidx0=gidx[:, :GIDX_C].copy(),
                        i32=i32, wcl=wcl, smat=smat))

    meta = dict(node2win=node2win, node2slot=node2slot,
                win_core=win_core, win_local=win_local, n_edges=meta_edges)
    return xp2, xr.copy(), ins, meta


# ---------------------------------------------------------------- device side

_PROG = None


def _build_program():
    import concourse.bacc as bacc
    import concourse.tile as tile
    from concourse import mybir, library_config, bass

    dt = mybir.dt
    Alu = mybir.AluOpType
    Act = mybir.ActivationFunctionType

    nc = bacc.Bacc("TRN2", target_bir_lowering=False, debug=False,
                   num_devices=NCORES, num_swdge_queues=NQ)

    xp2_d = nc.dram_tensor("xp2", [N // 2, ELEM2], dt.bfloat16, kind="ExternalInput")
    xrow_d = nc.dram_tensor("xrow", [N, DIM], dt.bfloat16, kind="ExternalInput")
    gidx0_d = nc.dram_tensor("gidx0", [P, GIDX_C], dt.int16, kind="ExternalInput")
    gidx_d = nc.dram_tensor("gidx", [P, NKC * GIDX_C], dt.int16, kind="ExternalInput")
    i32_d = nc.dram_tensor("i32", [P, max(1, NKC * NIC * IBK)], dt.int32,
                           kind="ExternalInput")
    smat_d = nc.dram_tensor("smat", [P, SLOTS], dt.bfloat16, kind="ExternalInput")
    wcl_d = nc.dram_tensor("wcl", [P, NBLK], dt.float32, kind="ExternalInput")
    coef_d = nc.dram_tensor("coef", [P, NCOEF], dt.float32, kind="ExternalInput")
    corr_d = nc.dram_tensor("corr", [1, 1], dt.float32, kind="ExternalInput")
    pbb_d = nc.dram_tensor("pbb", [P, 1], dt.float32, kind="ExternalInput")
    w0_d = nc.dram_tensor("w0", [SC, SC], dt.bfloat16, kind="ExternalInput")
    w1_d = nc.dram_tensor("w1", [VC, VC], dt.bfloat16, kind="ExternalInput")
    w1f_d = nc.dram_tensor("w1f", [VC, VC], dt.float32, kind="ExternalInput")
    identb_d = nc.dram_tensor("identb", [P, P], dt.bfloat16, kind="ExternalInput")
    gamr_d = nc.dram_tensor("gamr", [P, SC], dt.float32, kind="ExternalInput")
    betr_d = nc.dram_tensor("betr", [P, SC], dt.float32, kind="ExternalInput")
    onesc_d = nc.dram_tensor("onesc", [P, 1], dt.float32, kind="ExternalInput")
    onesr_d = nc.dram_tensor("onesr", [1, P], dt.float32, kind="ExternalInput")
    out_d = nc.dram_tensor("out", [OUTROWS, DIM], dt.float32, kind="ExternalOutput")

    s1 = float(1.0 / np.sqrt(VC) / E)

    with tile.TileContext(nc, num_cores=NCORES) as tc:
        import contextlib
        with contextlib.ExitStack() as ctx:
            consts = ctx.enter_context(tc.tile_pool(name="consts", bufs=1))
            gbuf = ctx.enter_context(tc.tile_pool(name="gbuf", bufs=1))
            gather = ctx.enter_context(tc.tile_pool(name="gather", bufs=2))
            ipool = ctx.enter_context(tc.tile_pool(name="ipool", bufs=2))
            spool = ctx.enter_context(tc.tile_pool(name="spool", bufs=2))
            obp = ctx.enter_context(tc.tile_pool(name="obp", bufs=2))
            gcp = ctx.enter_context(tc.tile_pool(name="gcp", bufs=6))
            ocp = ctx.enter_context(tc.tile_pool(name="ocp", bufs=6))
            zpool = ctx.enter_context(tc.tile_pool(name="zpool", bufs=3))
            lnp = ctx.enter_context(tc.tile_pool(name="lnp", bufs=4))
            psG = ctx.enter_context(tc.tile_pool(name="psG", bufs=2, space="PSUM"))
            psT = ctx.enter_context(tc.tile_pool(name="psT", bufs=2, space="PSUM"))
            psT2 = ctx.enter_context(tc.tile_pool(name="psT2", bufs=2, space="PSUM"))
            psE = ctx.enter_context(tc.tile_pool(name="psE", bufs=1, space="PSUM"))
            dram = ctx.enter_context(tc.tile_pool(name="dram", bufs=1, space="DRAM"))

            nc.gpsimd.load_library(library_config.mlp)

            # ---- constant loads (gidx0 first: the kc=0 gather needs only it)
            gidx0 = consts.tile([P, GIDX_C], dt.int16)
            nc.sync.dma_start(out=gidx0[:], in_=gidx0_d[:])
            i32 = consts.tile([P, max(1, NKC * NIC * IBK)], dt.int32)
            nc.sync.dma_start(out=i32[:], in_=i32_d[:])
            gidx = consts.tile([P, NKC * GIDX_C], dt.int16)
            nc.sync.dma_start(out=gidx[:], in_=gidx_d[:])
            wcl = consts.tile([P, NBLK], dt.float32)
            nc.sync.dma_start(out=wcl[:], in_=wcl_d[:])
            coef = consts.tile([P, NCOEF], dt.float32)
            nc.sync.dma_start(out=coef[:], in_=coef_d[:])
            corrt = consts.tile([1, 1], dt.float32)
            nc.sync.dma_start(out=corrt[:], in_=corr_d[:])
            pbb = consts.tile([P, 1], dt.float32)
            nc.sync.dma_start(out=pbb[:], in_=pbb_d[:])
            w0sb = consts.tile([SC, SC], dt.bfloat16)
            nc.sync.dma_start(out=w0sb[:], in_=w0_d[:])
            w1sb = consts.tile([VC, VC], dt.bfloat16)
            nc.sync.dma_start(out=w1sb[:], in_=w1_d[:])
            w1f = consts.tile([VC, VC], dt.float32)
            nc.sync.dma_start(out=w1f[:], in_=w1f_d[:])
            identb = consts.tile([P, P], dt.bfloat16)
            nc.sync.dma_start(out=identb[:], in_=identb_d[:])
            gamr = consts.tile([P, SC], dt.float32)
            nc.sync.dma_start(out=gamr[:], in_=gamr_d[:])
            betr = consts.tile([P, SC], dt.float32)
            nc.sync.dma_start(out=betr[:], in_=betr_d[:])
            onesc = consts.tile([P, 1], dt.float32)
            nc.sync.dma_start(out=onesc[:], in_=onesc_d[:])
            onesr = consts.tile([1, P], dt.float32)
            nc.sync.dma_start(out=onesr[:], in_=onesr_d[:])

            # ---- z-phase: Clenshaw of deg-11 Chebyshev (emitted up front;
            # runs on Vector while the first gathers proceed)
            w2c = gbuf.tile([P, NBLK], dt.float32)
            nc.vector.tensor_scalar(out=w2c[:], in0=wcl[:], scalar1=2.0,
                                    scalar2=None, op0=Alu.mult)
            b1 = zpool.tile([P, NBLK], dt.float32, tag="zb")
            nc.vector.tensor_scalar(out=b1[:], in0=wcl[:], scalar1=0.0,
                                    scalar2=coef[:, NCOEF - 1:NCOEF],
                                    op0=Alu.mult, op1=Alu.add)
            b2 = zpool.tile([P, NBLK], dt.float32, tag="zb")
            nc.vector.memset(b2[:], 0.0)
            for k in range(NCOEF - 2, 0, -1):
                t = zpool.tile([P, NBLK], dt.float32, tag="zt")
                nc.vector.tensor_tensor(out=t[:], in0=w2c[:], in1=b1[:],
                                        op=Alu.mult)
                bn = zpool.tile([P, NBLK], dt.float32, tag="zb")
                nc.vector.scalar_tensor_tensor(
                    out=bn[:], in0=t[:], scalar=coef[:, k:k + 1], in1=b2[:],
                    op0=Alu.add, op1=Alu.subtract)
                b2, b1 = b1, bn
            tf = zpool.tile([P, NBLK], dt.float32, tag="zt")
            nc.vector.tensor_tensor(out=tf[:], in0=wcl[:], in1=b1[:], op=Alu.mult)
            uz = gbuf.tile([P, NBLK], dt.float32)
            nc.vector.scalar_tensor_tensor(
                out=uz[:], in0=tf[:], scalar=coef[:, 0:1], in1=b2[:],
                op0=Alu.add, op1=Alu.subtract)
            zscr = gbuf.tile([P, NBLK], dt.float32)
            zsum = gbuf.tile([P, 1], dt.float32)

            ewc1 = gbuf.tile([P, 1], dt.float32)
            w1ew = gbuf.tile([VC, VC], dt.bfloat16)
            outv = out_d[:].rearrange("(w p) d -> p w d", p=P)

            def process_chunk(pkc, gpk):
                """Fused mix(+transpose-back) + LN + SiLU + store for chunk
                pkc.  Requires w1ew (ew-folded W1) to be emitted already."""
                och = ocp.tile([P, GW, DIM], dt.float32, tag="och", name="och")
                for wi in range(GW):
                    lo = wi * P
                    t2pk = psT2.tile([P, DIM], dt.float32, tag="t2pk", name="t2pk")
                    nc.tensor.matmul(t2pk[:, 0:SC], gpk[:, 0, lo:lo + P],
                                     w0sb[:], start=True, stop=True)
                    for j in range(3):
                        nc.tensor.matmul(t2pk[:, SC + VC * j:SC + VC * (j + 1)],
                                         gpk[0:VC, 1 + j, lo:lo + P],
                                         w1ew[:], start=True, stop=True)
                    nc.scalar.copy(out=och[:, wi, :], in_=t2pk[:])

                ob0 = och[:, :, 0:SC]
                mu = lnp.tile([P, GW], dt.float32, tag="mu", name="mu")
                nc.vector.tensor_reduce(out=mu[:], in_=ob0,
                                        axis=mybir.AxisListType.X, op=Alu.add)
                mu2 = lnp.tile([P, GW], dt.float32, tag="mu2", name="mu2")
                nc.vector.tensor_scalar(out=mu2[:], in0=mu[:],
                                        scalar1=float(1.0 / SC), scalar2=None,
                                        op0=Alu.mult)
                cen = lnp.tile([P, GW, SC], dt.float32, tag="cen", name="cen")
                nc.vector.tensor_tensor(
                    out=cen[:], in0=ob0,
                    in1=mu2[:].unsqueeze(2).to_broadcast([P, GW, SC]),
                    op=Alu.subtract)
                sq = lnp.tile([P, GW, SC], dt.float32, tag="lnt", name="sq")
                nc.vector.tensor_tensor(out=sq[:], in0=cen[:], in1=cen[:],
                                        op=Alu.mult)
                varb = lnp.tile([P, GW], dt.float32, tag="mu", name="varb")
                nc.vector.tensor_reduce(out=varb[:], in_=sq[:],
                                        axis=mybir.AxisListType.X, op=Alu.add)
                vb2 = lnp.tile([P, GW], dt.float32, tag="mu2", name="vb2")
                nc.vector.tensor_scalar(out=vb2[:], in0=varb[:],
                                        scalar1=float(1.0 / SC), scalar2=float(EPS),
                                        op0=Alu.mult, op1=Alu.add)
                sdb = lnp.tile([P, GW], dt.float32, tag="mu", name="sdb")
                nc.scalar.activation(out=sdb[:], in_=vb2[:], func=Act.Sqrt)
                rsb = lnp.tile([P, GW], dt.float32, tag="mu3", name="rsb")
                nc.vector.reciprocal(out=rsb[:], in_=sdb[:])
                t1b_ = lnp.tile([P, GW, SC], dt.float32, tag="lnt", name="t1b_")
                nc.vector.tensor_tensor(
                    out=t1b_[:], in0=cen[:],
                    in1=rsb[:].unsqueeze(2).to_broadcast([P, GW, SC]),
                    op=Alu.mult)
                t2b_ = lnp.tile([P, GW, SC], dt.float32, tag="lnt", name="t2b_")
                nc.vector.tensor_tensor(
                    out=t2b_[:], in0=t1b_[:],
                    in1=gamr[:].unsqueeze(1).to_broadcast([P, GW, SC]),
                    op=Alu.mult)
                t3b_ = lnp.tile([P, GW, SC], dt.float32, tag="lnt", name="t3b_")
                nc.vector.tensor_tensor(
                    out=t3b_[:], in0=t2b_[:],
                    in1=betr[:].unsqueeze(1).to_broadcast([P, GW, SC]),
                    op=Alu.add)
                nc.scalar.activation(out=ob0, in_=t3b_[:], func=Act.Silu)
                nc.scalar.dma_start(out=outv[:, pkc * GW:(pkc + 1) * GW, :],
                                    in_=och[:])

            pending = []
            prev = None
            for kc in range(NKC):
                sload = spool.tile([P, GW * BLKW, P], dt.bfloat16, tag="sload")
                nc.sync.dma_start(
                    out=sload[:],
                    in_=smat_d[:, kc * GW * BLKW * P:(kc + 1) * GW * BLKW * P])
                # indirect calls first: the dynamic ring drains while the
                # dma_gather desc-gen runs on the Pool Q7
                xi = []
                for ci in range(NIC):
                    xt = ipool.tile([P, IBK * DIM], dt.bfloat16, tag=f"xi{ci}")
                    c0 = (kc * NIC + ci) * IBK
                    nc.gpsimd.indirect_dma_start(
                        out=xt[:], out_offset=None, in_=xrow_d[:, :],
                        in_offset=bass.IndirectOffsetOnAxis(
                            ap=i32[:, c0:c0 + IBK], axis=0))
                    xi.append(xt)
                xgs = []
                for q in range(NQ):
                    xgq = gather.tile([P, GBQ, ELEM2], dt.bfloat16, tag=f"xg{q}")
                    src_t = gidx0 if kc == 0 else gidx
                    c0 = (0 if kc == 0 else kc * GIDX_C) + q * GIDX_CQ
                    nc.gpsimd.dma_gather(
                        xgq[:], xp2_d[:, :], src_t[:, c0:c0 + GIDX_CQ],
                        GIDX_NQ, GIDX_NQ, ELEM2, single_packet=False,
                        queue_num=q)
                    xgs.append(xgq)

                obw = obp.tile([P, GW, DIM], dt.bfloat16, tag="obw")
                gpk = gcp.tile([SC, 1 + 3, GW * P], dt.bfloat16, tag="gpk")
                gpos = {gb: j for j, gb in enumerate(_GBLOCKS)}
                ipos = {ib: j for j, ib in enumerate(_IBLOCKS)}
                for wi in range(GW):
                    pat = _WIN_PATTERNS[(kc * GW + wi) % 3]
                    gps = psG.tile([P, DIM], dt.float32, tag="gps")
                    for b in range(BLKW):
                        ty = pat[b]
                        if ty == "I":
                            j = ipos[(wi, b)]
                            ci, sub = j // IBK, j % IBK
                            operand = xi[ci][:, sub * DIM:(sub + 1) * DIM]
                        else:
                            j = gpos[(wi, b)]
                            sl = slice(0, DIM) if ty == "E" else \
                                slice(H2, H2 + DIM)
                            operand = xgs[j // GBQ][:, j % GBQ, sl]
                        nc.tensor.matmul(
                            gps[:], sload[:, wi * BLKW + b, :], operand,
                            start=(b == 0), stop=(b == BLKW - 1))
                    nc.scalar.copy(out=obw[:, wi, :], in_=gps[:])
                    # transpose to channel-major
                    tpk = psT.tile([SC, 4, P], dt.bfloat16, tag="tpk")
                    nc.tensor.transpose(out=tpk[:, 0, :], in_=obw[:, wi, 0:SC],
                                        identity=identb[:])
                    for j in range(3):
                        nc.tensor.transpose(
                            out=tpk[0:VC, 1 + j, :],
                            in_=obw[:, wi, SC + VC * j:SC + VC * (j + 1)],
                            identity=identb[:])
                    nc.scalar.copy(out=gpk[:, :, wi * P:(wi + 1) * P], in_=tpk[:])

                if kc == 1:
                    # z-phase tail after kc0/1 copies flowed through Scalar
                    nc.scalar.activation(out=zscr[:], in_=uz[:], func=Act.Sigmoid,
                                         bias=pbb[:, 0:1], accum_out=zsum[:, 0:1])
                    ewt = psE.tile([P, 2], dt.float32, tag="ewt", name="ewt")
                    nc.tensor.matmul(ewt[0:1, 0:1], onesc[:], zsum[:],
                                     start=True, stop=True)
                    zc8 = gbuf.tile([1, 8], dt.float32)
                    nc.vector.tensor_scalar(
                        out=zc8[:], in0=ewt[0:1, 0:1].to_broadcast([1, 8]),
                        scalar1=corrt[0:1, 0:1], scalar2=None, op0=Alu.subtract)
                    arin = dram.tile([1, 8], dt.float32)
                    arout = dram.tile([1, 8], dt.float32)
                    nc.sync.dma_start(out=arin[:], in_=zc8[:])
                    nc.gpsimd.collective_compute(
                        "AllReduce", Alu.add, replica_groups=[list(range(NCORES))],
                        ins=[arin.opt()], outs=[arout.opt()])

                if kc == 3:
                    ewsb = gbuf.tile([1, 8], dt.float32)
                    nc.sync.dma_start(out=ewsb[:], in_=arout[:])
                    ewg = gbuf.tile([1, 1], dt.float32)
                    nc.vector.tensor_scalar(out=ewg[:], in0=ewsb[0:1, 0:1],
                                            scalar1=s1, scalar2=None, op0=Alu.mult)
                    nc.tensor.matmul(ewt[:, 1:2], onesr[:], ewg[:],
                                     start=True, stop=True)
                    nc.scalar.copy(out=ewc1[:], in_=ewt[:, 1:2])
                    w1s = gbuf.tile([VC, VC], dt.float32)
                    nc.vector.tensor_scalar(out=w1s[:], in0=w1f[:],
                                            scalar1=ewc1[0:VC, 0:1],
                                            scalar2=None, op0=Alu.mult)
                    nc.vector.tensor_copy(out=w1ew[:], in_=w1s[:])
                    for pkc, pgpk in pending:
                        process_chunk(pkc, pgpk)
                    pending.clear()

                if kc < 3:
                    pending.append((kc, gpk))
                else:
                    if prev is not None:
                        process_chunk(*prev)
                    prev = (kc, gpk)

            process_chunk(*prev)

    nc.compile()
    return nc


def _get_program():
    global _PROG
    if _PROG is None:
        _PROG = _build_program()
    return _PROG


# ---------------------------------------------------------------- entry point

def kernel(**inputs):
    from concourse.bass_utils import run_bass_kernel_spmd

    x = np.asarray(inputs["x"], f32)
    pos = np.asarray(inputs["pos"], f32)
    ei = np.asarray(inputs["edge_index"])
    src = ei[0].astype(np.int64)
    dst = ei[1].astype(np.int64)

    xp2, xrow, cores, meta = _stage(x, pos, src, dst)

    cent = np.asarray(inputs["rbf_centers"], np.float64).reshape(-1)
    wid = np.asarray(inputs["rbf_widths"], np.float64).reshape(-1)
    pw = np.asarray(inputs["edge_proj_w"], np.float64).reshape(-1)
    pb = float(np.asarray(inputs["edge_proj_b"]).reshape(-1)[0])
    coefs = _fit_poly(cent, wid, pw)
    sig_pb = 1.0 / (1.0 + np.exp(-pb))

    common = dict(
        xp2=xp2,
        xrow=xrow,
        coef=np.tile(coefs[None, :], (P, 1)).astype(f32),
        pbb=np.full((P, 1), pb, f32),
        w0=np.asarray(inputs["W0"], f32).astype(bf16),
        w1=np.asarray(inputs["W1"], f32).astype(bf16),
        w1f=np.asarray(inputs["W1"], f32),
        identb=np.eye(P, dtype=bf16),
        gamr=np.tile(np.asarray(inputs["ln_gamma"], f32).reshape(1, SC), (P, 1)),
        betr=np.tile(np.asarray(inputs["ln_beta"], f32).reshape(1, SC), (P, 1)),
        onesc=np.ones((P, 1), f32),
        onesr=np.ones((1, P), f32),
    )
    in_maps = []
    for r in range(NCORES):
        cd = cores[r]
        n_empty = SLOTS - meta["n_edges"][r]
        in_maps.append(dict(
            common, gidx=cd["gidx"], gidx0=cd["gidx0"], i32=cd["i32"],
            wcl=cd["wcl"], smat=cd["smat"],
            corr=np.array([[n_empty * sig_pb]], f32)))

    nc = _get_program()
    trace = bool(int(os.environ.get("KERNEL_TRACE", "0")))
    res = run_bass_kernel_spmd(nc, in_maps, core_ids=list(range(NCORES)),
                               trace=trace)
    kernel.last_results = res

    # assemble full output
    out_full = np.zeros((N, DIM), f32)
    col_map = np.arange(DIM)
    for jj in range(3):
        for cc in range(VC):
            col_map[SC + 3 * cc + jj] = SC + VC * jj + cc
    n2w, n2s = meta["node2win"], meta["node2slot"]
    wc, wl = meta["win_core"], meta["win_local"]
    for r in range(NCORES):
        o = res.results[r]["out"]                      # [W*P, DIM]
        nodes = np.nonzero(wc == r)[0]
        rows = wl[nodes] * P + n2s[nodes]
        out_full[nodes] = o[rows][:, col_map]
    return out_full
